# revision 1
# baseline (speedup 1.0000x reference)
"""Trainium2 Bass kernel for nn_DecoderBlock (B=16,T=J=448,C=1024,H=16).

Sharding: data-parallel over batch, 2 samples per core on 8 cores. No
collectives. Each core runs the full decoder block on its 2 samples.

Layout strategy per sample:
  - LayerNorm in natural layout [T_p, C_f] (free-dim stats via bn_stats).
  - x transposed on PE -> xT [C_p, T_f] tiles; all big matmuls contract C (or
    4C) on partitions.
  - q^T,k^T produced directly in [HD_p, T_f] head layout; v in natural
    [T_p, C_f] augmented with a ones column per head so the attention row-sums
    (softmax denominators) fall out of the same PE matmul that computes S~@v.
  - S^T = k @ q^T per (head, key-chunk); causal handled by restricting the
    q-range per chunk plus one 128x128 additive mask on the diagonal block.
  - softmax normalization applied per-partition while copying O out of PSUM.
"""

import numpy as np
import ml_dtypes

import bass_rust
import concourse.bass as bass
import concourse.mybir as mybir
import concourse.tile as tile
from concourse.bass_utils import run_bass_kernel_spmd
from concourse.vector_clock import ScopedClock

F32 = mybir.dt.float32
AF = mybir.ActivationFunctionType
OP = mybir.AluOpType

B, T, J, C, H = 16, 448, 448, 1024, 16
HD = C // H
N_CORES = 8
BPC = B // N_CORES          # samples per core
SCALE = C ** (-0.5)
NEG = -1e9
EPS = 1e-5

# float32r ("reduced" fp32) runs the PE at full rate for N>=256 moving dims,
# carries ~13-bit precision, and -- crucially -- matmuls with f32r inputs stay
# SELF-LOADING (no separate Ldweights instruction on the PE stream, unlike
# bf16 which the legalizer splits into Ldweights+Matmult pairs).
F32R = mybir.dt.float32r
MM_DT = mybir.dt.float16      # x-path, weights, S, proj
AV_DT = mybir.dt.float16      # attention probabilities @ values
H_DT = mybir.dt.float16       # FFN hidden (hT) and W2
MM_NP = np.float16
H_NP = np.float16

TCH = [(0, 128), (128, 128), (256, 128), (384, 64)]   # (row0, rows) T-chunks
NCC = C // 128                                        # 8 C-chunks


class _TC(tile.TileContext):
    """TileContext whose final drain splits its semaphore waits across
    multiple single-wait Drain instructions (this walrus build rejects >1
    sync-wait per instruction)."""

    def _drain_and_barrier(self, tick_clock, wait_clock):
        nc = self.nc
        drain_inst = nc.sync.drain()
        wait_clock.add_sem_waits(
            drain_inst.ins, ScopedClock({None: tick_clock.global_clock})
        )
        si = drain_inst.ins.sync_info
        if si is not None and len(si.on_wait) > 1:
            waits = list(si.on_wait)
            drain_inst.ins.sync_info = bass_rust.SyncInfo(
                on_wait=waits[:1], on_update=list(si.on_update)
            )
            for w in waits[1:]:
                extra = nc.sync.drain()
                extra.ins.sync_info = bass_rust.SyncInfo(on_wait=[w], on_update=[])
        nc.all_engine_barrier()
        assert self.sems is not None
        popped = nc._tile_sem_poison_stack.pop()
        assert popped is self._sem_poison
        nc.clear_and_free_semaphores(list(self.sems.allocated().values()))
        nc.all_engine_barrier()


def _split_multi_waits(nc):
    """This walrus build accepts at most one sync-wait per instruction; move
    extra waits onto single-wait NoOps inserted just before, on the same
    engine (same semantics: the engine stalls on each wait in order)."""
    for f in nc.m.functions:
        for bb in f.blocks:
            new = []
            for inst in bb.instructions:
                si = getattr(inst, "sync_info", None)
                if si is not None and len(si.on_wait) > 1:
                    waits = list(si.on_wait)
                    for k, w in enumerate(waits[:-1]):
                        nop = mybir.InstNoOp(
                            name=f"{inst.name}-w{k}", ins=[], outs=[]
                        )
                        nop.engine = inst.engine
                        nop.sync_info = bass_rust.SyncInfo(
                            on_wait=[w], on_update=[]
                        )
                        new.append(nop)
                    inst.sync_info = bass_rust.SyncInfo(
                        on_wait=[waits[-1]], on_update=list(si.on_update)
                    )
                new.append(inst)
            bb.instructions[:] = new


def _build(affine, biases, reps=1):
    """Emit the Bass program. affine: apply LN gain/bias tensors. biases:
    apply proj/ffn output biases."""
    nc = bass.Bass()

    tgt_d = nc.dram_tensor("tgt", (BPC, T, C), F32, kind="ExternalInput")
    src_d = nc.dram_tensor("src", (BPC, J, C), F32, kind="ExternalInput")
    pad_d = nc.dram_tensor("pad", (BPC, 128, 4), F32, kind="ExternalInput")
    wqk_d = nc.dram_tensor("wqk", (16, 128, NCC, 128), MM_DT, kind="ExternalInput")
    wv_d = nc.dram_tensor("wv", (128, NCC, C), MM_DT, kind="ExternalInput")
    wproj_d = nc.dram_tensor("wproj", (128, NCC, C), MM_DT, kind="ExternalInput")
    ewk_d = nc.dram_tensor("ewk", (8, 128, NCC, 128), MM_DT, kind="ExternalInput")
    ewv_d = nc.dram_tensor("ewv", (128, NCC, C), MM_DT, kind="ExternalInput")
    ewq_d = nc.dram_tensor("ewq", (8, 128, NCC, 128), MM_DT, kind="ExternalInput")
    ewproj_d = nc.dram_tensor("ewproj", (128, NCC, C), MM_DT, kind="ExternalInput")
    w1_d = nc.dram_tensor("w1", (32, 128, NCC, 128), MM_DT, kind="ExternalInput")
    w2_d = nc.dram_tensor("w2", (32, 128, C), H_DT, kind="ExternalInput")
    b1_d = nc.dram_tensor("b1", (128, 32), F32, kind="ExternalInput")
    diag_d = nc.dram_tensor("diag", (128, 128), F32, kind="ExternalInput")
    ident_d = nc.dram_tensor("ident", (128, 128), MM_DT, kind="ExternalInput")
    if affine:
        lng_d = nc.dram_tensor("lng", (3, 128, C), F32, kind="ExternalInput")
        lnb_d = nc.dram_tensor("lnb", (3, 128, C), F32, kind="ExternalInput")
    if biases:
        pb_d = nc.dram_tensor("pb", (3, 128, C), F32, kind="ExternalInput")
    out_d = nc.dram_tensor("out", (BPC, T, C), F32, kind="ExternalOutput")

    # the rarely-used general paths (LN affine / proj biases) pin extra
    # broadcast constants in SBUF; shrink double-buffering to make room.
    lean = affine or biases
    with _TC(nc) as tc:
        import contextlib
        with contextlib.ExitStack() as ctx:
            consts = ctx.enter_context(tc.tile_pool(name="consts", bufs=1))
            small = ctx.enter_context(tc.tile_pool(name="small", bufs=6))
            tgtp = ctx.enter_context(tc.tile_pool(name="tgtp", bufs=2 if lean else 3))
            lnxp = ctx.enter_context(tc.tile_pool(name="lnxp", bufs=5 if lean else 8))
            xtp = ctx.enter_context(tc.tile_pool(name="xtp", bufs=8 if lean else 10))
            x2tp = ctx.enter_context(tc.tile_pool(name="x2tp", bufs=8))
            xstp = ctx.enter_context(tc.tile_pool(name="xstp", bufs=8))
            qkp = ctx.enter_context(tc.tile_pool(name="qkp", bufs=8 if lean else 10))
            vp = ctx.enter_context(tc.tile_pool(name="vp", bufs=5 if lean else 8))
            stp = ctx.enter_context(tc.tile_pool(name="stp", bufs=6 if lean else 10))
            otp = ctx.enter_context(tc.tile_pool(name="otp", bufs=9 if lean else 10))
            htp = ctx.enter_context(tc.tile_pool(name="htp", bufs=32))
            resp = ctx.enter_context(tc.tile_pool(name="resp", bufs=4))
            rtgp = ctx.enter_context(tc.tile_pool(name="rtgp", bufs=2))
            osbp = ctx.enter_context(tc.tile_pool(name="osbp", bufs=5 if lean else 8))
            wlhs = ctx.enter_context(tc.tile_pool(name="wlhs", bufs=3 if lean else 5))
            wrhs = ctx.enter_context(tc.tile_pool(name="wrhs", bufs=4))
            w2p = ctx.enter_context(tc.tile_pool(name="w2p", bufs=2 if lean else 3))
            ppA = ctx.enter_context(tc.tile_pool(name="ppA", bufs=4, space="PSUM"))
            ppO = ctx.enter_context(tc.tile_pool(name="ppO", bufs=2, space="PSUM"))
            ppT = ctx.enter_context(tc.tile_pool(name="ppT", bufs=2, space="PSUM"))

            ident = consts.tile([128, 128], MM_DT, tag="ident", name="ident")
            nc.sync.dma_start(out=ident, in_=ident_d[:])
            diag_t = consts.tile([128, 128], F32, tag="diag", name="diag")
            nc.sync.dma_start(out=diag_t, in_=diag_d[:])
            b1_t = consts.tile([128, 32], F32, tag="b1", name="b1")
            nc.sync.dma_start(out=b1_t, in_=b1_d[:])
            eps_t = consts.tile([128, 1], F32, tag="eps", name="eps")
            nc.vector.memset(eps_t, EPS)

            if affine:
                lng_t = [consts.tile([128, C], F32, tag=f"lng{i}", name="tile") for i in range(3)]
                lnb_t = [consts.tile([128, C], F32, tag=f"lnb{i}", name="tile") for i in range(3)]
                for i in range(3):
                    nc.sync.dma_start(out=lng_t[i], in_=lng_d[i])
                    nc.sync.dma_start(out=lnb_t[i], in_=lnb_d[i])
            if biases:
                pb_t = [consts.tile([128, C], F32, tag=f"pb{i}", name="tile") for i in range(3)]
                for i in range(3):
                    nc.sync.dma_start(out=pb_t[i], in_=pb_d[i])

            def ln_tile(src_ap, rt, ln_idx):
                """LayerNorm one [rt, C] natural tile -> MM_DT tile."""
                stats = small.tile([128, 2, 6], F32, tag="stats", name="stats")
                mv = small.tile([128, 2], F32, tag="mv", name="mv")
                rstd = small.tile([128, 1], F32, tag="rstd", name="rstd")
                nmr = small.tile([128, 1], F32, tag="nmr", name="nmr")
                for sg in range(2):
                    nc.vector.bn_stats(
                        out=stats[:rt, sg], in_=src_ap[:, sg * 512:(sg + 1) * 512]
                    )
                nc.vector.bn_aggr(out=mv[:rt], in_=stats[:rt])
                nc.scalar.activation(
                    out=rstd[:rt], in_=mv[:rt, 1:2], func=AF.Ln,
                    bias=eps_t[:rt], scale=1.0,
                )
                nc.scalar.activation(
                    out=rstd[:rt], in_=rstd[:rt], func=AF.Exp,
                    bias=0.0, scale=-0.5,
                )
                nc.vector.tensor_scalar(
                    out=nmr[:rt], in0=mv[:rt, 0:1], scalar1=rstd[:rt],
                    scalar2=-1.0, op0=OP.mult, op1=OP.mult,
                )
                x_t = lnxp.tile([128, C], MM_DT, tag="lnx", name="lnx")
                nc.scalar.activation(
                    out=x_t[:rt], in_=src_ap, func=AF.Identity,
                    bias=nmr[:rt], scale=rstd[:rt],
                )
                if affine:
                    nc.vector.tensor_tensor(
                        x_t[:rt], x_t[:rt], lng_t[ln_idx][:rt], OP.mult
                    )
                    nc.vector.tensor_tensor(
                        x_t[:rt], x_t[:rt], lnb_t[ln_idx][:rt], OP.add
                    )
                return x_t

            def transpose_1024(x_tiles, pool, tag):
                """[T,C] natural MM_DT tiles -> 8 xT tiles [128, 448]."""
                xT = []
                for cc in range(NCC):
                    ps = ppT.tile([128, 448], MM_DT, tag="psT", name="psT")
                    for ti, (r0, rt) in enumerate(TCH):
                        nc.tensor.transpose(
                            ps[:, r0:r0 + rt],
                            x_tiles[ti][:rt, cc * 128:(cc + 1) * 128],
                            ident[:rt, :rt],
                        )
                    t = pool.tile([128, 448], MM_DT, tag=tag, name="tile")
                    nc.vector.tensor_copy(out=t[:], in_=ps[:])
                    xT.append(t)
                return xT

            def mm_qk_one(w_dram, m, xT, tag):
                """One transposed projection m-tile [128, 448]."""
                wt = wlhs.tile([128, NCC, 128], MM_DT, tag="wlhs", name="wlhs")
                nc.sync.dma_start(out=wt, in_=w_dram[m])
                ps = ppA.tile([128, 512], F32, tag="psA", name="psA448")[:, :448]
                for cc in range(NCC):
                    nc.tensor.matmul(
                        ps, wt[:, cc], xT[cc],
                        start=(cc == 0), stop=(cc == NCC - 1),
                    )
                t = qkp.tile([128, 448], MM_DT, tag=tag, name="qkT")
                nc.vector.tensor_copy(out=t[:], in_=ps[:])
                return t

            def mm_v2(wv_dram, xT):
                """v in natural layout augmented with ones columns:
                4 tiles [128, 16*65] (per head: 64 v dims + ones)."""
                v_aug = []
                for ti, (r0, rt) in enumerate(TCH):
                    va = vp.tile([128, 16, 65], AV_DT, tag="vaug", name="vaug")
                    nc.vector.memset(va[:, :, 64:65], 1.0)
                    v_aug.append(va)
                for half in range(2):
                    wts = []
                    for cq in range(2):
                        wt = wrhs.tile([128, 4, 512], MM_DT, tag="wrhs",
                                       name="wrhs")
                        nc.sync.dma_start(
                            out=wt,
                            in_=wv_dram[:, cq * 4:(cq + 1) * 4,
                                        half * 512:(half + 1) * 512],
                        )
                        wts.append(wt)
                    for ti, (r0, rt) in enumerate(TCH):
                        ps = ppA.tile([128, 512], F32, tag="psA", name="psA")
                        for cc in range(NCC):
                            nc.tensor.matmul(
                                ps[:rt], xT[cc][:, r0:r0 + rt],
                                wts[cc // 4][:, cc % 4],
                                start=(cc == 0), stop=(cc == NCC - 1),
                            )
                        nc.vector.tensor_copy(
                            out=v_aug[ti][:rt, half * 8:(half + 1) * 8, 0:64],
                            in_=ps[:rt].rearrange("p (h d) -> p h d", h=8),
                        )
                return v_aug

            def attention(mk_qk, v_aug, causal, pad_t):
                """-> o_sb: 4 natural tiles [128, C] AV_DT, softmax-normalized
                per-partition while copying O out of PSUM."""
                o_sb = [osbp.tile([128, C], AV_DT, tag="osb", name="osb")
                        for _ in TCH]
                qkT = {m: mk_qk(m) for m in range(8)}
                for h in range(H):
                    qi, off = h // 2, 64 * (h % 2)
                    qTt, kTt = qkT[qi]
                    # S^T chunks, exp -> SBUF
                    stx = []
                    for c, (k0, kr) in enumerate(TCH):
                        q0 = c * 128 if causal else 0
                        qw = 448 - q0
                        ps = ppA.tile([128, 512], F32, tag="psA", name="psA448")[:, :448]
                        nc.tensor.matmul(
                            ps[:kr, :qw],
                            kTt[off:off + 64, k0:k0 + kr],
                            qTt[off:off + 64, q0:448],
                            start=True, stop=True,
                        )
                        if causal:
                            nc.vector.tensor_tensor(
                                ps[:kr, 0:kr], ps[:kr, 0:kr],
                                diag_t[:kr, :kr], OP.add,
                            )
                        st = stp.tile([128, 448], AV_DT, tag="stx", name="stx")
                        nc.scalar.activation(
                            out=st[:kr, :qw], in_=ps[:kr, :qw], func=AF.Exp,
                            bias=pad_t[:kr, c:c + 1] if pad_t is not None else 0.0,
                            scale=SCALE,
                        )
                        stx.append(st)
                    # O = S~ @ [v | 1] accumulated per q-chunk; col 64 = sums
                    po = ppO.tile([128, 4 * 65], F32, tag="psO", name="psO")
                    for qc, (q0, qr) in enumerate(TCH):
                        cmax = qc if causal else 3
                        for c in range(cmax + 1):
                            kr = TCH[c][1]
                            col0 = (qc - c) * 128 if causal else q0
                            nc.tensor.matmul(
                                po[:qr, qc * 65:(qc + 1) * 65],
                                stx[c][:kr, col0:col0 + qr],
                                v_aug[c][:kr, h],
                                start=(c == 0), stop=(c == cmax),
                            )
                    rec = small.tile([128, 4], F32, tag="rec", name="rec")
                    for qc, (q0, qr) in enumerate(TCH):
                        nc.vector.reciprocal(
                            out=rec[:qr, qc:qc + 1],
                            in_=po[:qr, qc * 65 + 64:qc * 65 + 65],
                        )
                        nc.vector.tensor_scalar_mul(
                            out=o_sb[qc][:qr, h * 64:(h + 1) * 64],
                            in0=po[:qr, qc * 65:qc * 65 + 64],
                            scalar1=rec[:qr, qc:qc + 1],
                        )
                return o_sb

            def proj_residual(o_sb, wp_dram, res_tiles, resid_from_dram, b,
                              bias_idx):
                """result = resid + O @ Wp (+bias). o_sb transposed on PE."""
                oT = []
                for cc in range(NCC):
                    ps = ppT.tile([128, 448], AV_DT, tag="psT", name="psT")
                    for ti, (r0, rt) in enumerate(TCH):
                        nc.tensor.transpose(
                            ps[:, r0:r0 + rt],
                            o_sb[ti][:rt, cc * 128:(cc + 1) * 128],
                            ident[:rt, :rt],
                        )
                    t = otp.tile([128, 448], MM_DT, tag="oT", name="oT")
                    nc.vector.tensor_copy(out=t[:], in_=ps[:])
                    oT.append(t)
                for half in range(2):
                    wts = []
                    for cq in range(2):
                        wt = wrhs.tile([128, 4, 512], MM_DT, tag="wrhs",
                                       name="wrhs")
                        nc.sync.dma_start(
                            out=wt,
                            in_=wp_dram[:, cq * 4:(cq + 1) * 4,
                                        half * 512:(half + 1) * 512],
                        )
                        wts.append(wt)
                    for ti, (r0, rt) in enumerate(TCH):
                        ps = ppA.tile([128, 512], F32, tag="psA", name="psA")
                        for cc in range(NCC):
                            nc.tensor.matmul(
                                ps[:rt], oT[cc][:, r0:r0 + rt],
                                wts[cc // 4][:, cc % 4],
                                start=(cc == 0), stop=(cc == NCC - 1),
                            )
                        hs = slice(half * 512, (half + 1) * 512)
                        if resid_from_dram:
                            rtg = rtgp.tile([128, 512], F32, tag="rtg", name="rtg")
                            nc.sync.dma_start(
                                out=rtg[:rt], in_=tgt_d[b, r0:r0 + rt, hs]
                            )
                            nc.vector.tensor_tensor(
                                res_tiles[ti][:rt, hs], ps[:rt], rtg[:rt], OP.add
                            )
                        else:
                            nc.vector.tensor_tensor(
                                res_tiles[ti][:rt, hs], ps[:rt],
                                res_tiles[ti][:rt, hs], OP.add,
                            )
                        if biases:
                            nc.vector.tensor_tensor(
                                res_tiles[ti][:rt, hs], res_tiles[ti][:rt, hs],
                                pb_t[bias_idx][:rt, hs], OP.add,
                            )

            for b in [bb for _ in range(reps) for bb in range(BPC)]:
                # ---- stage A: LN1(tgt) (== LN3(tgt) when affine is off) ----
                x1_tiles = []
                for ti, (r0, rt) in enumerate(TCH):
                    tt = tgtp.tile([128, C], F32, tag="tgt_nat", name="tgt_nat")
                    nc.sync.dma_start(out=tt[:rt], in_=tgt_d[b, r0:r0 + rt, :])
                    x1_tiles.append(ln_tile(tt[:rt], rt, 0))
                x1T = transpose_1024(x1_tiles, xtp, "x1T")

                pad_t = small.tile([128, 4], F32, tag="padt", name="padt")
                nc.sync.dma_start(out=pad_t, in_=pad_d[b])

                # ---- stage B/C: self-attention ----
                v_aug = mm_v2(wv_d, x1T)

                def mk_qk_self(m):
                    return (mm_qk_one(wqk_d, m, x1T, "qT"),
                            mm_qk_one(wqk_d[8:16], m, x1T, "kT"))

                oT1 = attention(mk_qk_self, v_aug, causal=True, pad_t=pad_t)
                res_tiles = [resp.tile([128, C], F32, tag="result", name="result") for _ in TCH]
                proj_residual(oT1, wproj_d, res_tiles, True, b, 0)

                # ---- stage D: cross-attention ----
                x2_tiles = [
                    ln_tile(res_tiles[ti][:rt], rt, 1)
                    for ti, (r0, rt) in enumerate(TCH)
                ]
                x2T = transpose_1024(x2_tiles, x2tp, "x2T")
                xs_tiles = []
                for ti, (r0, rt) in enumerate(TCH):
                    st_ = tgtp.tile([128, C], F32, tag="tgt_nat", name="tgt_nat")
                    nc.sync.dma_start(out=st_[:rt], in_=src_d[b, r0:r0 + rt, :])
                    xs_tiles.append(ln_tile(st_[:rt], rt, 1))
                xsT = transpose_1024(xs_tiles, xstp, "xsT")

                v2_aug = mm_v2(ewv_d, xsT)

                def mk_qk_cross(m):
                    return (mm_qk_one(ewq_d, m, x2T, "qT"),
                            mm_qk_one(ewk_d, m, xsT, "kT"))

                oT2 = attention(mk_qk_cross, v2_aug, causal=False, pad_t=None)
                proj_residual(oT2, ewproj_d, res_tiles, False, b, 1)

                # ---- stage E: FFN on ln3(tgt) ----
                if affine:
                    x3_tiles = []
                    for ti, (r0, rt) in enumerate(TCH):
                        tt = tgtp.tile([128, C], F32, tag="tgt_nat", name="tgt_nat")
                        nc.sync.dma_start(out=tt[:rt], in_=tgt_d[b, r0:r0 + rt, :])
                        x3_tiles.append(ln_tile(tt[:rt], rt, 2))
                    x3T = transpose_1024(x3_tiles, xtp, "x1T")
                else:
                    x3T = x1T

                hT = []
                for m in range(32):
                    wt = wlhs.tile([128, NCC, 128], MM_DT, tag="wlhs", name="wlhs")
                    nc.sync.dma_start(out=wt, in_=w1_d[m])
                    ps = ppA.tile([128, 512], F32, tag="psA", name="psA448")[:, :448]
                    for cc in range(NCC):
                        nc.tensor.matmul(
                            ps, wt[:, cc], x3T[cc],
                            start=(cc == 0), stop=(cc == NCC - 1),
                        )
                    ht = htp.tile([128, 448], H_DT, tag="hT", name="hT")
                    nc.scalar.activation(
                        out=ht[:], in_=ps[:], func=AF.Relu,
                        bias=b1_t[:, m:m + 1], scale=1.0,
                    )
                    hT.append(ht)

                for half in range(2):
                    hs = slice(half * 512, (half + 1) * 512)
                    for grp in range(2):
                        chunks = TCH[grp * 2:(grp + 1) * 2]
                        psy = [ppA.tile([128, 512], F32, tag="psA", name="psA")
                               for _ in chunks]
                        for mq in range(8):
                            wt = w2p.tile([128, 4, 512], H_DT, tag="w2",
                                          name="w2")
                            nc.sync.dma_start(
                                out=wt,
                                in_=w2_d[mq * 4:(mq + 1) * 4, :, hs]
                                .rearrange("m p n -> p m n"),
                            )
                            for i4 in range(4):
                                m = mq * 4 + i4
                                for i, (r0, rt) in enumerate(chunks):
                                    nc.tensor.matmul(
                                        psy[i][:rt], hT[m][:, r0:r0 + rt],
                                        wt[:, i4],
                                        start=(m == 0), stop=(m == 31),
                                    )
                        for i, (r0, rt) in enumerate(chunks):
                            ti = grp * 2 + i
                            nc.vector.tensor_tensor(
                                res_tiles[ti][:rt, hs], psy[i][:rt],
                                res_tiles[ti][:rt, hs], OP.add,
                            )
                            if biases:
                                nc.vector.tensor_tensor(
                                    res_tiles[ti][:rt, hs],
                                    res_tiles[ti][:rt, hs],
                                    pb_t[2][:rt, hs], OP.add,
                                )
                for ti, (r0, rt) in enumerate(TCH):
                    nc.sync.dma_start(
                        out=out_d[b, r0:r0 + rt, :], in_=res_tiles[ti][:rt]
                    )
    _split_multi_waits(nc)
    return nc


def _pack_lhsT(w):
    """[C, M] -> [M//128, 128, C//128, 128] so an lhsT m-tile is one
    contiguous DMA with 2KB-per-partition lines."""
    Cdim, M = w.shape
    return np.ascontiguousarray(
        w.reshape(Cdim // 128, 128, M // 128, 128).transpose(2, 1, 0, 3)
    )


def _pack_rhs(w):
    """[K, N] -> [128, K//128, N]."""
    K, N = w.shape
    return np.ascontiguousarray(w.reshape(K // 128, 128, N).transpose(1, 0, 2))


def prepare(inputs):
    inp = {k: np.asarray(v) for k, v in inputs.items()}

    affine = not all(
        np.all(inp[f"ln{i}_g"] == 1.0) and np.all(inp[f"ln{i}_b"] == 0.0)
        for i in (1, 2, 3)
    )
    biases = not (
        np.all(inp["sa_bproj"] == 0.0)
        and np.all(inp["ed_bproj"] == 0.0)
        and np.all(inp["ff_b2"] == 0.0)
    )

    f32 = np.float32
    qkv = inp["sa_wqkv"].astype(f32)
    shared = {
        "wqk": _pack_lhsT(qkv[:, :2048]).astype(MM_NP),
        "wv": _pack_rhs(qkv[:, 2048:]).astype(MM_NP),
        "wproj": _pack_rhs(inp["sa_wproj"].astype(f32)).astype(MM_NP),
        "ewk": _pack_lhsT(inp["ed_wkv"][:, :1024].astype(f32)).astype(MM_NP),
        "ewv": _pack_rhs(inp["ed_wkv"][:, 1024:].astype(f32)).astype(MM_NP),
        "ewq": _pack_lhsT(inp["ed_wq"].astype(f32)).astype(MM_NP),
        "ewproj": _pack_rhs(inp["ed_wproj"].astype(f32)).astype(MM_NP),
        "w1": _pack_lhsT(inp["ff_w1"].astype(f32)).astype(MM_NP),
        "w2": np.ascontiguousarray(
            inp["ff_w2"].astype(f32).reshape(32, 128, C)
        ).astype(H_NP),
        "b1": np.ascontiguousarray(
            inp["ff_b1"].astype(f32).reshape(32, 128).T
        ),
        "diag": np.where(
            np.arange(128)[:, None] > np.arange(128)[None, :], NEG, 0.0
        ).astype(f32),
        "ident": np.eye(128, dtype=f32).astype(MM_NP),
    }
    if affine:
        shared["lng"] = np.stack(
            [np.tile(inp[f"ln{i}_g"].astype(f32), (128, 1), name="tile") for i in (1, 2, 3)]
        )
        shared["lnb"] = np.stack(
            [np.tile(inp[f"ln{i}_b"].astype(f32), (128, 1), name="tile") for i in (1, 2, 3)]
        )
    if biases:
        shared["pb"] = np.stack(
            [
                np.tile(inp["sa_bproj"].astype(f32), (128, 1), name="tile"),
                np.tile(inp["ed_bproj"].astype(f32), (128, 1), name="tile"),
                np.tile(inp["ff_b2"].astype(f32), (128, 1), name="tile"),
            ]
        )

    pad_add = np.where(inp["tgt_padding_mask"], NEG, 0.0).astype(f32)  # [B,T]
    pad_full = np.zeros((B, 512), f32)
    pad_full[:, :T] = pad_add
    pad_packed = np.ascontiguousarray(
        pad_full.reshape(B, 4, 128).transpose(0, 2, 1)
    )  # [B,128,4]

    tgt = inp["tgt"].astype(f32)
    src = inp["src"].astype(f32)

    nc = _build(affine, biases)
    in_maps = []
    for c in range(N_CORES):
        s = slice(c * BPC, (c + 1) * BPC)
        m = dict(shared)
        m["tgt"] = np.ascontiguousarray(tgt[s])
        m["src"] = np.ascontiguousarray(src[s])
        m["pad"] = np.ascontiguousarray(pad_packed[s])
        in_maps.append(m)
    return nc, in_maps


def kernel(**inputs):
    nc, in_maps = prepare(inputs)
    res = run_bass_kernel_spmd(nc, in_maps, core_ids=list(range(N_CORES)))
    out = np.concatenate([res.results[c]["out"] for c in range(N_CORES)], axis=0)
    return out.astype(np.float32)

